# revision 1
# baseline (speedup 1.0000x reference)
"""DGI (Deep Graph Infomax) Trainium2 kernel.

Strategy (8 NeuronCores, one shared SPMD program):
  - Nodes sharded by destination: core c owns dst nodes [c*N/8, (c+1)*N/8).
  - xw = x @ W computed replicated on every core in fp16 (DMA-transpose
    loads + PE matmuls), written to per-core DRAM.
  - GCN aggregation: edges (incl. self-loops, symmetric norm precomputed on
    host) are sorted by (dst tile, src<32768), padded to 128-edge tiles with
    (idx=0, w=0, dstl=-1).  Each 128-edge tile: dma_gather of xw rows
    (int16 indices; hi half gathered from an offset AP), weighted one-hot
    S_Tw built on DVE (is_equal vs iota, scaled by norm), PE matmul
    S_Tw.T @ gathered accumulated into the dst tile's PSUM.
  - PReLU(agg + b) -> z tiles; z1 kept in SBUF, z2 streamed.
  - summary = sigmoid(mean(z1)): DVE tree column-sum + ones-matmul,
    1KB AllReduce across the 8 cores, sigmoid on ACT.
  - wsum = disc_W @ summary via PE (host passes disc_W.T); broadcast via
    K=1 matmul; pos/neg = z . wsum via fused tensor_tensor_reduce.
  - Per-core [128, DT] outputs; host unshards/concatenates.
"""

import os

import numpy as np

_P = 128
_LO = 32768
_C = 8


def _build_streams(sidx, ed, ew, C, NS, DT):
    """Build per-core gather/weight/dstl streams with a shared tile structure.

    sidx: source index per edge (already permuted for the corrupted pass)
    ed:   destination node per edge
    ew:   edge weight (symmetric norm) per edge
    Returns (idx_sbuf [C,128,n_et*8] i16, w_sbuf [C,128,n_et] f32,
             dl_sbuf [C,128,n_et] f16, Tmax [DT,2] int, off_tiles [DT,2] int,
             n_et)
    """
    core = ed // NS
    ldst = ed - core * NS
    dt = ldst // _P
    dstl = ldst % _P
    cls = (sidx >= _LO).astype(np.int64)

    gid = (core * DT + dt) * 2 + cls
    NG = C * DT * 2
    cnt = np.bincount(gid, minlength=NG).reshape(C, DT, 2)
    T = -(-cnt // _P)
    Tmax = T.max(axis=0)  # shared structure across cores
    flat = Tmax.reshape(-1)
    off_tiles = np.concatenate([[0], np.cumsum(flat)[:-1]]).reshape(DT, 2)
    n_et = int(flat.sum())

    order = np.argsort(gid, kind="stable")
    sorted_gid = gid[order]
    g_starts = np.concatenate(
        [[0], np.cumsum(np.bincount(sorted_gid, minlength=NG))[:-1]]
    )
    rank = np.arange(order.size) - g_starts[sorted_gid]
    g_dt = (sorted_gid // 2) % DT
    g_cls = sorted_gid % 2
    pos = off_tiles[g_dt, g_cls] * _P + rank
    core_s = sorted_gid // (DT * 2)

    L = n_et * _P
    idx16 = np.zeros((C, L), np.int16)
    wv = np.zeros((C, L), np.float32)
    dl = np.full((C, L), -1.0, np.float16)
    sidx_s = sidx[order]
    idx16[core_s, pos] = (sidx_s - g_cls * _LO).astype(np.int16)
    wv[core_s, pos] = ew[order]
    dl[core_s, pos] = dstl[order].astype(np.float16)

    idx_w = idx16.reshape(C, L // 16, 16).transpose(0, 2, 1)
    idx_sbuf = np.ascontiguousarray(np.tile(idx_w, (1, 8, 1)))
    w_sbuf = np.ascontiguousarray(wv.reshape(C, n_et, _P).transpose(0, 2, 1))
    dl_sbuf = np.ascontiguousarray(dl.reshape(C, n_et, _P).transpose(0, 2, 1))
    return idx_sbuf, w_sbuf, dl_sbuf, Tmax, off_tiles, n_et


def kernel(x, W, b, a, disc_W, edge_index, perm):
    import bass_rust
    import concourse.bacc as bacc
    import concourse.mybir as mybir
    import concourse.tile as tile
    from concourse.bass_utils import run_bass_kernel_spmd

    x = np.asarray(x)
    W = np.asarray(W)
    b = np.asarray(b, np.float32)
    a = np.asarray(a, np.float32)
    disc_W = np.asarray(disc_W, np.float32)
    ei = np.asarray(edge_index, np.int64)
    perm_np = np.asarray(perm, np.int64)

    N, F = x.shape
    H = W.shape[1]
    C = _C
    NS = N // C
    DT = -(-NS // _P)
    LAST = NS - (DT - 1) * _P  # valid rows of the last dst tile
    f16 = mybir.dt.float16
    f32 = mybir.dt.float32

    # ---- host preprocessing -------------------------------------------
    src = ei[0]
    dst = ei[1]
    deg = (np.bincount(dst, minlength=N) + 1.0).astype(np.float32)
    dinv = (1.0 / np.sqrt(deg)).astype(np.float32)
    loops = np.arange(N, dtype=np.int64)
    es = np.concatenate([src, loops])
    ed = np.concatenate([dst, loops])
    ew = dinv[es] * dinv[ed]
    es2 = perm_np[es]

    i1, w1, d1, T1, O1, n_et1 = _build_streams(es, ed, ew, C, NS, DT)
    i2, w2, d2, T2, O2, n_et2 = _build_streams(es2, ed, ew, C, NS, DT)

    x_f16 = np.ascontiguousarray(x.astype(np.float16))
    W_f16 = np.ascontiguousarray(W.astype(np.float16))
    dwT = np.ascontiguousarray(disc_W.T.astype(np.float32))
    iota_np = np.tile(np.arange(_P, dtype=np.float16)[None, :], (_P, 1))

    # ---- device program -----------------------------------------------
    nc = bacc.Bacc("TRN2", target_bir_lowering=False, debug=False, num_devices=C)

    t_x = nc.dram_tensor("x16", [N, F], f16, kind="ExternalInput")
    t_W = nc.dram_tensor("w16", [F, H], f16, kind="ExternalInput")
    t_b = nc.dram_tensor("bvec", [H], f32, kind="ExternalInput")
    t_a = nc.dram_tensor("avec", [1], f32, kind="ExternalInput")
    t_dwT = nc.dram_tensor("dwT", [H, H], f32, kind="ExternalInput")
    t_iota = nc.dram_tensor("iota", [_P, _P], f16, kind="ExternalInput")
    t_ident = nc.dram_tensor("ident_in", [_P, _P], f32, kind="ExternalInput")
    t_i1 = nc.dram_tensor("idx1", [_P, n_et1 * 8], mybir.dt.int16, kind="ExternalInput")
    t_w1 = nc.dram_tensor("wgt1", [_P, n_et1], f32, kind="ExternalInput")
    t_d1 = nc.dram_tensor("dstl1", [_P, n_et1], f16, kind="ExternalInput")
    t_i2 = nc.dram_tensor("idx2", [_P, n_et2 * 8], mybir.dt.int16, kind="ExternalInput")
    t_w2 = nc.dram_tensor("wgt2", [_P, n_et2], f32, kind="ExternalInput")
    t_d2 = nc.dram_tensor("dstl2", [_P, n_et2], f16, kind="ExternalInput")

    t_pos = nc.dram_tensor("pos_out", [_P, DT], f32, kind="ExternalOutput")
    t_neg = nc.dram_tensor("neg_out", [_P, DT], f32, kind="ExternalOutput")

    t_xw = nc.dram_tensor("xw", [N, H], f16)
    t_ar_in = nc.dram_tensor("ar_in", [H], f32)
    t_ar_out = nc.dram_tensor("ar_out", [H], f32, addr_space="Shared")

    CHUNK = 512  # phase-1 node rows per transposed load
    STAGE = int(os.environ.get("KERNEL_STAGE", "4"))

    with tile.TileContext(nc) as tc:
        import contextlib

        ctx = contextlib.ExitStack()
        consts = ctx.enter_context(tc.tile_pool(name="consts", bufs=1))
        ph1 = ctx.enter_context(tc.tile_pool(name="ph1", bufs=3))
        ph1ps = ctx.enter_context(tc.tile_pool(name="ph1ps", bufs=2, space="PSUM"))
        glo = ctx.enter_context(tc.tile_pool(name="glo", bufs=2))
        ghi = ctx.enter_context(tc.tile_pool(name="ghi", bufs=2))
        stp = ctx.enter_context(tc.tile_pool(name="stp", bufs=4))
        aggps = ctx.enter_context(tc.tile_pool(name="aggps", bufs=3, space="PSUM"))
        misc = ctx.enter_context(tc.tile_pool(name="misc", bufs=2))
        miscps = ctx.enter_context(tc.tile_pool(name="miscps", bufs=1, space="PSUM"))

        # ---- constants ----
        W0 = consts.tile([_P, H], f16, tag="W0")
        W1 = consts.tile([_P, H], f16, tag="W1")
        nc.sync.dma_start(W0[:], t_W[0:_P, :])
        nc.sync.dma_start(W1[:], t_W[_P : 2 * _P, :])
        iota_t = consts.tile([_P, _P], f16, tag="iota")
        nc.sync.dma_start(iota_t[:], t_iota[:])
        b_sb = consts.tile([1, H], f32, tag="b_sb")
        nc.sync.dma_start(b_sb[:], t_b[None, :])
        a_sb = consts.tile([1, 1], f32, tag="a_sb")
        nc.sync.dma_start(a_sb[:], t_a[None, :])
        dwT0 = consts.tile([_P, H], f32, tag="dwT0")
        dwT1 = consts.tile([_P, H], f32, tag="dwT1")
        nc.sync.dma_start(dwT0[:], t_dwT[0:_P, :])
        nc.sync.dma_start(dwT1[:], t_dwT[_P : 2 * _P, :])
        ones_row = consts.tile([1, _P], f32, tag="ones_row")
        nc.vector.memset(ones_row[:], 1.0)
        ones_col = consts.tile([_P, 1], f32, tag="ones_col")
        nc.vector.memset(ones_col[:], 1.0)

        # broadcasts via K=1 matmul
        bb_ps = miscps.tile([_P, H], f32, tag="mps")
        nc.tensor.matmul(bb_ps[:], ones_row[:], b_sb[:], start=True, stop=True)
        b_bc = consts.tile([_P, H], f32, tag="b_bc")
        nc.vector.tensor_copy(b_bc[:], bb_ps[:])
        ab_ps = miscps.tile([_P, 1], f32, tag="mps")
        nc.tensor.matmul(ab_ps[:], ones_row[:], a_sb[:], start=True, stop=True)
        a_bc = consts.tile([_P, 1], f32, tag="a_bc")
        nc.vector.tensor_copy(a_bc[:], ab_ps[:])

        # ---- stream loads ----
        i1_sb = consts.tile([_P, n_et1 * 8], mybir.dt.int16, tag="i1")
        w1_sb = consts.tile([_P, n_et1], f32, tag="w1")
        d1_sb = consts.tile([_P, n_et1], f16, tag="d1")
        nc.sync.dma_start(i1_sb[:], t_i1[:])
        nc.sync.dma_start(w1_sb[:], t_w1[:])
        nc.sync.dma_start(d1_sb[:], t_d1[:])
        i2_sb = consts.tile([_P, n_et2 * 8], mybir.dt.int16, tag="i2")
        w2_sb = consts.tile([_P, n_et2], f32, tag="w2")
        d2_sb = consts.tile([_P, n_et2], f16, tag="d2")
        nc.sync.dma_start(i2_sb[:], t_i2[:])
        nc.sync.dma_start(w2_sb[:], t_w2[:])
        nc.sync.dma_start(d2_sb[:], t_d2[:])

        # ---- phase 1: xw = x @ W (replicated) ----
        for r0 in range(0, N, CHUNK):
            rows = min(CHUNK, N - r0)
            xT0 = ph1.tile([_P, CHUNK], f16, tag="xT0")
            xT1 = ph1.tile([_P, CHUNK], f16, tag="xT1")
            nc.sync.dma_start_transpose(xT0[:, :rows], t_x[r0 : r0 + rows, 0:_P])
            nc.sync.dma_start_transpose(
                xT1[:, :rows], t_x[r0 : r0 + rows, _P : 2 * _P]
            )
            for o in range(0, rows, _P):
                m = min(_P, rows - o)
                ps = ph1ps.tile([_P, H], f32, tag="ph1ps")
                nc.tensor.matmul(
                    ps[:m, :], xT0[:, o : o + m], W0[:], start=True, stop=False
                )
                nc.tensor.matmul(
                    ps[:m, :], xT1[:, o : o + m], W1[:], start=False, stop=True
                )
                xw_sb = ph1.tile([_P, H], f16, tag="xw_sb")
                nc.any.tensor_copy(xw_sb[:m, :], ps[:m, :])
                nc.sync.dma_start(t_xw[r0 + o : r0 + o + m, :], xw_sb[:m, :])

        xw_lo = t_xw[0:_LO, :] if N > _LO else t_xw[:, :]
        xw_hi = t_xw[_LO:N, :] if N > _LO else None

        # ---- aggregation passes ----
        zbuf = consts.tile([_P, DT * H], f32, tag="zbuf")  # z1 persists

        def agg_pass(idx_sb, w_sb, dl_sb, Tm, Ot, z_consumer):
            for dti in range(DT):
                Tl, Th = int(Tm[dti, 0]), int(Tm[dti, 1])
                gl = gh = None
                if Tl:
                    o = int(Ot[dti, 0])
                    gl = glo.tile([_P, max_Tl, H], f16, tag="gl")
                    nc.gpsimd.dma_gather(
                        gl[:, :Tl, :],
                        xw_lo,
                        idx_sb[:, 8 * o : 8 * (o + Tl)],
                        Tl * _P,
                        Tl * _P,
                        H,
                        single_packet=(Tl * _P <= 1024),
                    )
                if Th:
                    o = int(Ot[dti, 1])
                    gh = ghi.tile([_P, max_Th, H], f16, tag="gh")
                    nc.gpsimd.dma_gather(
                        gh[:, :Th, :],
                        xw_hi,
                        idx_sb[:, 8 * o : 8 * (o + Th)],
                        Th * _P,
                        Th * _P,
                        H,
                        single_packet=(Th * _P <= 1024),
                    )
                ps = aggps.tile([_P, H], f32, tag="aggps")
                n_mm = Tl + Th
                k = 0
                for cls_i, (Tn, g, o0) in enumerate(
                    [(Tl, gl, int(Ot[dti, 0])), (Th, gh, int(Ot[dti, 1]))]
                ):
                    for j in range(Tn):
                        t = o0 + j
                        eq = stp.tile([_P, _P], f16, tag="eq")
                        nc.vector.tensor_tensor(
                            eq[:],
                            dl_sb[:, t : t + 1].to_broadcast([_P, _P]),
                            iota_t[:],
                            mybir.AluOpType.is_equal,
                        )
                        stw = stp.tile([_P, _P], f16, tag="stw")
                        nc.vector.tensor_scalar(
                            stw[:],
                            eq[:],
                            w_sb[:, t : t + 1],
                            None,
                            mybir.AluOpType.mult,
                        )
                        nc.tensor.matmul(
                            ps[:],
                            stw[:],
                            g[:, j, :],
                            start=(k == 0),
                            stop=(k == n_mm - 1),
                        )
                        k += 1
                z_consumer(dti, ps)

        # z1 consumer: bias + PReLU into persistent zbuf
        def z1_consume(dti, ps):
            zs = zbuf[:, dti * H : (dti + 1) * H]
            nc.vector.tensor_tensor(zs, ps[:], b_bc[:], mybir.AluOpType.add)
            t1 = misc.tile([_P, H], f32, tag="t1")
            nc.vector.tensor_scalar(
                t1[:], zs, 0.0, a_bc[:, 0:1],
                mybir.AluOpType.min, mybir.AluOpType.mult,
            )
            t2 = misc.tile([_P, H], f32, tag="t2")
            nc.vector.tensor_scalar(t2[:], zs, 0.0, None, mybir.AluOpType.max)
            nc.vector.tensor_tensor(zs, t1[:], t2[:], mybir.AluOpType.add)

        max_Tl = max(int(T1[:, 0].max()), int(T2[:, 0].max()), 1)
        max_Th = max(int(T1[:, 1].max()), int(T2[:, 1].max()), 1)

        pos_acc = consts.tile([_P, DT], f32, tag="pos_acc")
        neg_acc = consts.tile([_P, DT], f32, tag="neg_acc")
        nc.vector.memset(pos_acc[:], 0.0)
        nc.vector.memset(neg_acc[:], 0.0)

        if STAGE >= 2:
            agg_pass(i1_sb, w1_sb, d1_sb, T1, O1, z1_consume)

        # ---- summary: column sum of z1 over all nodes ----
        if STAGE >= 3:
            cacc = misc.tile([_P, H], f32, tag="cacc")
            nc.vector.tensor_copy(cacc[:], zbuf[:, 0:H])
            for dti in range(1, DT):
                rows = LAST if dti == DT - 1 else _P
                nc.vector.tensor_tensor(
                    cacc[:rows, :],
                    cacc[:rows, :],
                    zbuf[:rows, dti * H : (dti + 1) * H],
                    mybir.AluOpType.add,
                )
            cs_ps = miscps.tile([1, H], f32, tag="mps")
            nc.tensor.matmul(cs_ps[:], ones_col[:], cacc[:], start=True, stop=True)
            cs_sb = misc.tile([1, H], f32, tag="cs_sb")
            nc.vector.tensor_copy(cs_sb[:], cs_ps[:])
            nc.sync.dma_start(t_ar_in[None, :], cs_sb[:])
            nc.gpsimd.collective_compute(
                "AllReduce",
                mybir.AluOpType.add,
                replica_groups=[list(range(C))],
                ins=[t_ar_in[:]],
                outs=[t_ar_out[:]],
            )
            sums_sb = misc.tile([1, H], f32, tag="sums_sb")
            nc.sync.dma_start(sums_sb[:], t_ar_out[None, :])
            summ_sb = misc.tile([1, H], f32, tag="summ_sb")
            nc.scalar.activation(
                summ_sb[:], sums_sb[:], mybir.ActivationFunctionType.Sigmoid,
                scale=1.0 / N,
            )

            # ---- wsum = disc_W @ summary ----
            ident = consts.tile([_P, _P], f32, tag="ident")
            nc.sync.dma_start(ident[:], t_ident[:])
            sT = misc.tile([_P, 2], f32, tag="sT")
            for c_i in range(2):
                tp = miscps.tile([_P, _P], f32, tag="mps")
                nc.tensor.transpose(
                    tp[:, 0:1],
                    summ_sb[0:1, c_i * _P : (c_i + 1) * _P],
                    ident[0:1, 0:1],
                )
                nc.vector.tensor_copy(sT[:, c_i : c_i + 1], tp[:, 0:1])
            ws_ps = miscps.tile([1, H], f32, tag="mps")
            nc.tensor.matmul(ws_ps[:], sT[:, 0:1], dwT0[:], start=True, stop=False)
            nc.tensor.matmul(ws_ps[:], sT[:, 1:2], dwT1[:], start=False, stop=True)
            ws_sb = misc.tile([1, H], f32, tag="ws_sb")
            nc.vector.tensor_copy(ws_sb[:], ws_ps[:])
            wb_ps = miscps.tile([_P, H], f32, tag="mps")
            nc.tensor.matmul(wb_ps[:], ones_row[:], ws_sb[:], start=True, stop=True)
            wsum_bc = consts.tile([_P, H], f32, tag="wsum_bc")
            nc.vector.tensor_copy(wsum_bc[:], wb_ps[:])

        scratch = misc.tile([_P, H], f32, tag="scratch")

        # ---- z2 pass with inline neg dot ----
        def z2_consume(dti, ps):
            zt = misc.tile([_P, H], f32, tag="zt")
            nc.vector.tensor_tensor(zt[:], ps[:], b_bc[:], mybir.AluOpType.add)
            t1 = misc.tile([_P, H], f32, tag="t1")
            nc.vector.tensor_scalar(
                t1[:], zt[:], 0.0, a_bc[:, 0:1],
                mybir.AluOpType.min, mybir.AluOpType.mult,
            )
            t2 = misc.tile([_P, H], f32, tag="t2")
            nc.vector.tensor_scalar(t2[:], zt[:], 0.0, None, mybir.AluOpType.max)
            nc.vector.tensor_tensor(zt[:], t1[:], t2[:], mybir.AluOpType.add)
            nc.vector.tensor_tensor(
                scratch[:], zt[:], wsum_bc[:], mybir.AluOpType.mult
            )
            nc.vector.reduce_sum(
                neg_acc[:, dti : dti + 1], scratch[:], bass_rust.AxisListType.X
            )

        if STAGE >= 4:
            agg_pass(i2_sb, w2_sb, d2_sb, T2, O2, z2_consume)

            # ---- pos dots from persistent z1 ----
            for dti in range(DT):
                nc.vector.tensor_tensor(
                    scratch[:], zbuf[:, dti * H : (dti + 1) * H], wsum_bc[:],
                    mybir.AluOpType.mult,
                )
                nc.vector.reduce_sum(
                    pos_acc[:, dti : dti + 1], scratch[:], bass_rust.AxisListType.X
                )

        nc.sync.dma_start(t_pos[:], pos_acc[:])
        nc.sync.dma_start(t_neg[:], neg_acc[:])
        ctx.close()

    nc.compile()

    in_maps = []
    for c in range(C):
        in_maps.append(
            {
                "x16": x_f16,
                "w16": W_f16,
                "bvec": b,
                "avec": a,
                "dwT": dwT,
                "iota": iota_np,
                "ident_in": np.eye(_P, dtype=np.float32),
                "idx1": i1[c],
                "wgt1": w1[c],
                "dstl1": d1[c],
                "idx2": i2[c],
                "wgt2": w2[c],
                "dstl2": d2[c],
            }
        )

    if os.environ.get("KERNEL_SIM", "0") == "1":
        from concourse import bass_interp

        sim = bass_interp.MultiCoreSim(nc, C)
        for c in range(C):
            for k, v in in_maps[c].items():
                sim.cores[c].tensor(k)[:] = v
        sim.simulate()
        results = [
            {
                "pos_out": np.array(sim.cores[c].tensor("pos_out")),
                "neg_out": np.array(sim.cores[c].tensor("neg_out")),
            }
            for c in range(C)
        ]
    else:
        trace = os.environ.get("KERNEL_TRACE", "0") == "1"
        kw = {}
        if trace:
            kw["trace"] = True
        res = run_bass_kernel_spmd(nc, in_maps, core_ids=list(range(C)), **kw)
        kernel.last_result = res
        results = res.results

    pos = np.zeros(N, np.float32)
    neg = np.zeros(N, np.float32)
    for c in range(C):
        pos[c * NS : (c + 1) * NS] = results[c]["pos_out"].T.reshape(-1)[:NS]
        neg[c * NS : (c + 1) * NS] = results[c]["neg_out"].T.reshape(-1)[:NS]
    return pos, neg



# revision 4
# speedup vs baseline: 3.0289x; 3.0289x over previous
"""DGI (Deep Graph Infomax) Trainium2 kernel — v2.

Strategy (8 NeuronCores, one shared SPMD program):
  - Nodes sharded by destination: core c owns dst nodes [c*N/8, (c+1)*N/8).
  - Aggregate-then-multiply: z = PReLU((A_hat @ x) @ W + b).  The x-space
    aggregation needs no precomputed xw, so gathers start immediately.
  - Paired table tab[i] = [x[i] | x[perm[i]]] (f16, 1KB rows).  dma_gather
    cost is row-count-bound (~8ns/row, flat 256B..1KB), so one gather per
    edge fetches BOTH passes' source rows.  One shared weighted one-hot
    (fused is_equal+mult on DVE) and one PE matmul [128e,128d]^T@[128e,512]
    accumulate [agg1|agg2] per dst tile in PSUM.
  - Gathers round-robin over 4 SWDGE queues (~2x DMA overlap).
  - Epilogue per dst tile: PSUM->SBUF f16, 2+2 PE transposes, (aggT @ W)
    k-accumulated, bias+PReLU on DVE; z1/z2 tiles stored f16 in SBUF;
    z1 column-sums accumulate for the summary.
  - summary = sigmoid(mean over all z1) via ones-matmul column reduce +
    1KB AllReduce; wsum = disc_W @ summary on PE; broadcast via K=1 matmul.
  - pos/neg = z . wsum per tile on DVE; per-core [128, DT] outputs,
    host unshards.
"""

import os

import numpy as np

_P = 128
_LO = 32768
_C = 8


def _build_streams(sidx, ed, ew, C, NS, DT):
    """Per-core gather/weight/dstl streams with shared tile structure.

    Returns (idx_sbuf [C,128,n_et*8] i16, w_sbuf [C,128,n_et] f32,
             dl_sbuf [C,128,n_et] f32, Tmax [DT,2], off_tiles [DT,2], n_et)
    """
    core = ed // NS
    ldst = ed - core * NS
    dt = ldst // _P
    dstl = ldst % _P
    cls = (sidx >= _LO).astype(np.int64)

    gid = (core * DT + dt) * 2 + cls
    NG = C * DT * 2
    cnt = np.bincount(gid, minlength=NG).reshape(C, DT, 2)
    T = -(-cnt // _P)
    Tmax = T.max(axis=0)
    flat = Tmax.reshape(-1)
    off_tiles = np.concatenate([[0], np.cumsum(flat)[:-1]]).reshape(DT, 2)
    n_et = int(flat.sum())

    order = np.argsort(gid, kind="stable")
    sorted_gid = gid[order]
    g_starts = np.concatenate(
        [[0], np.cumsum(np.bincount(sorted_gid, minlength=NG))[:-1]]
    )
    rank = np.arange(order.size) - g_starts[sorted_gid]
    g_dt = (sorted_gid // 2) % DT
    g_cls = sorted_gid % 2
    pos = off_tiles[g_dt, g_cls] * _P + rank
    core_s = sorted_gid // (DT * 2)

    L = n_et * _P
    idx16 = np.zeros((C, L), np.int16)
    wv = np.zeros((C, L), np.float32)
    dl = np.full((C, L), -1.0, np.float32)
    sidx_s = sidx[order]
    idx16[core_s, pos] = (sidx_s - g_cls * _LO).astype(np.int16)
    wv[core_s, pos] = ew[order]
    dl[core_s, pos] = dstl[order].astype(np.float32)

    idx_w = idx16.reshape(C, L // 16, 16).transpose(0, 2, 1)
    idx_sbuf = np.ascontiguousarray(np.tile(idx_w, (1, 8, 1)))
    w_sbuf = np.ascontiguousarray(wv.reshape(C, n_et, _P).transpose(0, 2, 1))
    dl_sbuf = np.ascontiguousarray(dl.reshape(C, n_et, _P).transpose(0, 2, 1))
    return idx_sbuf, w_sbuf, dl_sbuf, Tmax, off_tiles, n_et


def kernel(x, W, b, a, disc_W, edge_index, perm):
    import bass_rust
    import concourse.bacc as bacc
    import concourse.mybir as mybir
    import concourse.tile as tile
    from concourse.bass_utils import run_bass_kernel_spmd

    x = np.asarray(x)
    W = np.asarray(W)
    b = np.asarray(b, np.float32)
    a = np.asarray(a, np.float32)
    disc_W = np.asarray(disc_W, np.float32)
    ei = np.asarray(edge_index, np.int64)
    perm_np = np.asarray(perm, np.int64)

    N, F = x.shape
    H = W.shape[1]
    C = _C
    NS = N // C
    DT = -(-NS // _P)
    LAST = NS - (DT - 1) * _P
    F2 = 2 * F  # paired row width
    f16 = mybir.dt.float16
    f32 = mybir.dt.float32
    NQ = 4  # SWDGE queues

    # ---- host preprocessing -------------------------------------------
    src = ei[0]
    dst = ei[1]
    deg = (np.bincount(dst, minlength=N) + 1.0).astype(np.float32)
    dinv = (1.0 / np.sqrt(deg)).astype(np.float32)
    loops = np.arange(N, dtype=np.int64)
    es = np.concatenate([src, loops])
    ed = np.concatenate([dst, loops])
    ew = dinv[es] * dinv[ed]

    i1, w1, d1, T1, O1, n_et = _build_streams(es, ed, ew, C, NS, DT)

    x_f16 = x.astype(np.float16)
    tab = np.ascontiguousarray(np.concatenate([x_f16, x_f16[perm_np]], axis=1))
    W_f16 = np.ascontiguousarray(W.astype(np.float16))
    dwT = np.ascontiguousarray(disc_W.T.astype(np.float32))
    iota_np = np.tile(np.arange(_P, dtype=np.float16)[None, :], (_P, 1))
    ident16_np = np.eye(_P, dtype=np.float16)

    max_Tl = max(int(T1[:, 0].max()), 1)
    max_Th = max(int(T1[:, 1].max()), 1)

    # ---- device program -----------------------------------------------
    nc = bacc.Bacc(
        "TRN2", target_bir_lowering=False, debug=False, num_devices=C,
        num_swdge_queues=NQ,
    )

    t_tab = nc.dram_tensor("tab", [N, F2], f16, kind="ExternalInput")
    t_W = nc.dram_tensor("w16", [F, H], f16, kind="ExternalInput")
    t_b = nc.dram_tensor("bvec", [H], f32, kind="ExternalInput")
    t_a = nc.dram_tensor("avec", [1], f32, kind="ExternalInput")
    t_dwT = nc.dram_tensor("dwT", [H, H], f32, kind="ExternalInput")
    t_iota = nc.dram_tensor("iota", [_P, _P], f16, kind="ExternalInput")
    t_ident = nc.dram_tensor("ident_in", [_P, _P], f32, kind="ExternalInput")
    t_ident16 = nc.dram_tensor("ident16", [_P, _P], f16, kind="ExternalInput")
    t_i1 = nc.dram_tensor("idx1", [_P, n_et * 8], mybir.dt.int16, kind="ExternalInput")
    t_w1 = nc.dram_tensor("wgt1", [_P, n_et], f32, kind="ExternalInput")
    t_d1 = nc.dram_tensor("dstl1", [_P, n_et], f32, kind="ExternalInput")

    t_pos = nc.dram_tensor("pos_out", [_P, DT], f32, kind="ExternalOutput")
    t_neg = nc.dram_tensor("neg_out", [_P, DT], f32, kind="ExternalOutput")

    t_ar_in = nc.dram_tensor("ar_in", [H], f32)
    t_ar_out = nc.dram_tensor("ar_out", [H], f32, addr_space="Shared")

    tab_lo = t_tab[0:_LO, :]
    tab_hi = t_tab[_LO:N, :]

    with tile.TileContext(nc) as tc:
        import contextlib

        ctx = contextlib.ExitStack()
        consts = ctx.enter_context(tc.tile_pool(name="consts", bufs=1))
        glo = ctx.enter_context(tc.tile_pool(name="glo", bufs=2))
        ghi = ctx.enter_context(tc.tile_pool(name="ghi", bufs=2))
        stp = ctx.enter_context(tc.tile_pool(name="stp", bufs=4))
        aggps = ctx.enter_context(tc.tile_pool(name="aggps", bufs=2, space="PSUM"))
        trps = ctx.enter_context(tc.tile_pool(name="trps", bufs=2, space="PSUM"))
        zps = ctx.enter_context(tc.tile_pool(name="zps", bufs=2, space="PSUM"))
        ep = ctx.enter_context(tc.tile_pool(name="ep", bufs=3))
        misc = ctx.enter_context(tc.tile_pool(name="misc", bufs=2))
        miscps = ctx.enter_context(tc.tile_pool(name="miscps", bufs=1, space="PSUM"))

        # ---- constants ----
        W0 = consts.tile([_P, H], f16, tag="W0")
        W1 = consts.tile([_P, H], f16, tag="W1")
        nc.sync.dma_start(W0[:], t_W[0:_P, :])
        nc.sync.dma_start(W1[:], t_W[_P : 2 * _P, :])
        iota_t = consts.tile([_P, _P], f16, tag="iota")
        nc.sync.dma_start(iota_t[:], t_iota[:])
        ident16 = consts.tile([_P, _P], f16, tag="ident16")
        nc.sync.dma_start(ident16[:], t_ident16[:])
        b_sb = consts.tile([1, H], f32, tag="b_sb")
        nc.sync.dma_start(b_sb[:], t_b[None, :])
        a_sb = consts.tile([1, 1], f32, tag="a_sb")
        nc.sync.dma_start(a_sb[:], t_a[None, :])
        dwT0 = consts.tile([_P, H], f32, tag="dwT0")
        dwT1 = consts.tile([_P, H], f32, tag="dwT1")
        nc.sync.dma_start(dwT0[:], t_dwT[0:_P, :])
        nc.sync.dma_start(dwT1[:], t_dwT[_P : 2 * _P, :])
        ones_row = consts.tile([1, _P], f32, tag="ones_row")
        nc.vector.memset(ones_row[:], 1.0)
        ones_col = consts.tile([_P, 1], f32, tag="ones_col")
        nc.vector.memset(ones_col[:], 1.0)

        # broadcasts via K=1 matmul
        bb_ps = miscps.tile([_P, H], f32, tag="mps")
        nc.tensor.matmul(bb_ps[:], ones_row[:], b_sb[:], start=True, stop=True)
        b_bc = consts.tile([_P, H], f32, tag="b_bc")
        nc.vector.tensor_copy(b_bc[:], bb_ps[:])
        ab_ps = miscps.tile([_P, 1], f32, tag="mps")
        nc.tensor.matmul(ab_ps[:], ones_row[:], a_sb[:], start=True, stop=True)
        a_bc = consts.tile([_P, 1], f32, tag="a_bc")
        nc.vector.tensor_copy(a_bc[:], ab_ps[:])

        # ---- stream loads ----
        i1_sb = consts.tile([_P, n_et * 8], mybir.dt.int16, tag="i1")
        w1_sb = consts.tile([_P, n_et], f32, tag="w1")
        d1_sb = consts.tile([_P, n_et], f32, tag="d1")
        nc.sync.dma_start(i1_sb[:], t_i1[:])
        nc.sync.dma_start(w1_sb[:], t_w1[:])
        nc.sync.dma_start(d1_sb[:], t_d1[:])

        # ---- persistent z tiles + summary accumulator ----
        zbuf1 = consts.tile([_P, DT * H], f16, tag="zbuf1")
        zbuf2 = consts.tile([_P, DT * H], f16, tag="zbuf2")
        cacc = consts.tile([_P, H], f32, tag="cacc")
        nc.vector.memset(cacc[:], 0.0)

        qctr = [0]

        # ---- main sweep: one pass over dst tiles serves both encodings --
        for dti in range(DT):
            Tl, Th = int(T1[dti, 0]), int(T1[dti, 1])
            gl = gh = None
            if Tl:
                o = int(O1[dti, 0])
                gl = glo.tile([_P, max_Tl, F2], f16, tag="gl")
                nc.gpsimd.dma_gather(
                    gl[:, :Tl, :],
                    tab_lo,
                    i1_sb[:, 8 * o : 8 * (o + Tl)],
                    Tl * _P,
                    Tl * _P,
                    F2,
                    single_packet=(Tl * _P <= 1024),
                    queue_num=qctr[0] % NQ,
                )
                qctr[0] += 1
            if Th:
                o = int(O1[dti, 1])
                gh = ghi.tile([_P, max_Th, F2], f16, tag="gh")
                nc.gpsimd.dma_gather(
                    gh[:, :Th, :],
                    tab_hi,
                    i1_sb[:, 8 * o : 8 * (o + Th)],
                    Th * _P,
                    Th * _P,
                    F2,
                    single_packet=(Th * _P <= 1024),
                    queue_num=qctr[0] % NQ,
                )
                qctr[0] += 1

            ps = aggps.tile([_P, F2], f32, tag="aggps")
            n_mm = Tl + Th
            k = 0
            for Tn, g, o0 in ((Tl, gl, int(O1[dti, 0])), (Th, gh, int(O1[dti, 1]))):
                for j in range(Tn):
                    t = o0 + j
                    stw = stp.tile([_P, _P], f16, tag="stw")
                    nc.vector.tensor_scalar(
                        stw[:],
                        iota_t[:],
                        d1_sb[:, t : t + 1],
                        w1_sb[:, t : t + 1],
                        mybir.AluOpType.is_equal,
                        mybir.AluOpType.mult,
                    )
                    nc.tensor.matmul(
                        ps[:],
                        stw[:],
                        g[:, j, :],
                        start=(k == 0),
                        stop=(k == n_mm - 1),
                    )
                    k += 1

            # epilogue: agg [128, 512] = [agg1 | agg2] -> z1, z2 tiles
            agg_sb = ep.tile([_P, F2], f16, tag="agg_sb")
            nc.any.tensor_copy(agg_sb[:], ps[:])
            zp = zps.tile([_P, F2], f32, tag="zp")
            for k4 in range(4):
                tp = trps.tile([_P, _P], f16, tag="trps")
                nc.tensor.transpose(
                    tp[:], agg_sb[:, k4 * _P : (k4 + 1) * _P], ident16[:]
                )
                aggT = ep.tile([_P, _P], f16, tag="aggT")
                nc.any.tensor_copy(aggT[:], tp[:])
                half = 0 if k4 < 2 else 1
                Wk = W0 if (k4 % 2) == 0 else W1
                nc.tensor.matmul(
                    zp[:, half * H : (half + 1) * H], aggT[:], Wk[:],
                    start=((k4 % 2) == 0), stop=((k4 % 2) == 1),
                )

            for half, zbuf, is_z1 in ((0, zbuf1, True), (1, zbuf2, False)):
                zadd = misc.tile([_P, H], f32, tag="zadd")
                nc.vector.tensor_tensor(
                    zadd[:], zp[:, half * H : (half + 1) * H], b_bc[:],
                    mybir.AluOpType.add,
                )
                t1 = misc.tile([_P, H], f32, tag="t1")
                nc.vector.tensor_scalar(
                    t1[:], zadd[:], 0.0, a_bc[:, 0:1],
                    mybir.AluOpType.min, mybir.AluOpType.mult,
                )
                t2 = misc.tile([_P, H], f32, tag="t2")
                nc.vector.tensor_scalar(
                    t2[:], zadd[:], 0.0, None, mybir.AluOpType.max
                )
                zf = misc.tile([_P, H], f32, tag="zf")
                nc.vector.tensor_tensor(zf[:], t1[:], t2[:], mybir.AluOpType.add)
                nc.any.tensor_copy(zbuf[:, dti * H : (dti + 1) * H], zf[:])
                if is_z1:
                    rows = LAST if dti == DT - 1 else _P
                    nc.vector.tensor_tensor(
                        cacc[:rows, :], cacc[:rows, :], zf[:rows, :],
                        mybir.AluOpType.add,
                    )

        # ---- summary + AllReduce ----
        cs_ps = miscps.tile([1, H], f32, tag="mps")
        nc.tensor.matmul(cs_ps[:], ones_col[:], cacc[:], start=True, stop=True)
        cs_sb = misc.tile([1, H], f32, tag="cs_sb")
        nc.vector.tensor_copy(cs_sb[:], cs_ps[:])
        nc.sync.dma_start(t_ar_in[None, :], cs_sb[:])
        nc.gpsimd.collective_compute(
            "AllReduce",
            mybir.AluOpType.add,
            replica_groups=[list(range(C))],
            ins=[t_ar_in[:]],
            outs=[t_ar_out[:]],
        )
        sums_sb = misc.tile([1, H], f32, tag="sums_sb")
        nc.sync.dma_start(sums_sb[:], t_ar_out[None, :])
        summ_sb = misc.tile([1, H], f32, tag="summ_sb")
        nc.scalar.activation(
            summ_sb[:], sums_sb[:], mybir.ActivationFunctionType.Sigmoid,
            scale=1.0 / N,
        )

        # ---- wsum = disc_W @ summary ----
        ident = consts.tile([_P, _P], f32, tag="ident")
        nc.sync.dma_start(ident[:], t_ident[:])
        sT = misc.tile([_P, 2], f32, tag="sT")
        for c_i in range(2):
            tp = miscps.tile([_P, _P], f32, tag="mps")
            nc.tensor.transpose(
                tp[:, 0:1],
                summ_sb[0:1, c_i * _P : (c_i + 1) * _P],
                ident[0:1, 0:1],
            )
            nc.vector.tensor_copy(sT[:, c_i : c_i + 1], tp[:, 0:1])
        ws_ps = miscps.tile([1, H], f32, tag="mps")
        nc.tensor.matmul(ws_ps[:], sT[:, 0:1], dwT0[:], start=True, stop=False)
        nc.tensor.matmul(ws_ps[:], sT[:, 1:2], dwT1[:], start=False, stop=True)
        ws_sb = misc.tile([1, H], f32, tag="ws_sb")
        nc.vector.tensor_copy(ws_sb[:], ws_ps[:])
        wb_ps = miscps.tile([_P, H], f32, tag="mps")
        nc.tensor.matmul(wb_ps[:], ones_row[:], ws_sb[:], start=True, stop=True)
        wsum_bc = consts.tile([_P, H], f16, tag="wsum_bc")
        nc.vector.tensor_copy(wsum_bc[:], wb_ps[:])

        # ---- pos/neg dots ----
        pos_acc = consts.tile([_P, DT], f32, tag="pos_acc")
        neg_acc = consts.tile([_P, DT], f32, tag="neg_acc")
        scratch = misc.tile([_P, H], f16, tag="scratch")
        for zbuf, acc in ((zbuf1, pos_acc), (zbuf2, neg_acc)):
            for dti in range(DT):
                nc.vector.tensor_tensor(
                    scratch[:], zbuf[:, dti * H : (dti + 1) * H], wsum_bc[:],
                    mybir.AluOpType.mult,
                )
                nc.vector.reduce_sum(
                    acc[:, dti : dti + 1], scratch[:], bass_rust.AxisListType.X
                )

        nc.sync.dma_start(t_pos[:], pos_acc[:])
        nc.sync.dma_start(t_neg[:], neg_acc[:])
        ctx.close()

    nc.compile()

    in_maps = []
    for c in range(C):
        in_maps.append(
            {
                "tab": tab,
                "w16": W_f16,
                "bvec": b,
                "avec": a,
                "dwT": dwT,
                "iota": iota_np,
                "ident_in": np.eye(_P, dtype=np.float32),
                "ident16": ident16_np,
                "idx1": i1[c],
                "wgt1": w1[c],
                "dstl1": d1[c],
            }
        )

    if os.environ.get("KERNEL_SIM", "0") == "1":
        from concourse import bass_interp

        sim = bass_interp.MultiCoreSim(nc, C)
        for c in range(C):
            for k, v in in_maps[c].items():
                sim.cores[c].tensor(k)[:] = v
        sim.simulate()
        results = [
            {
                "pos_out": np.array(sim.cores[c].tensor("pos_out")),
                "neg_out": np.array(sim.cores[c].tensor("neg_out")),
            }
            for c in range(C)
        ]
    else:
        trace = os.environ.get("KERNEL_TRACE", "0") == "1"
        kw = {}
        if trace:
            kw["trace"] = True
        res = run_bass_kernel_spmd(nc, in_maps, core_ids=list(range(C)), **kw)
        kernel.last_result = res
        results = res.results

    pos = np.zeros(N, np.float32)
    neg = np.zeros(N, np.float32)
    for c in range(C):
        pos[c * NS : (c + 1) * NS] = results[c]["pos_out"].T.reshape(-1)[:NS]
        neg[c * NS : (c + 1) * NS] = results[c]["neg_out"].T.reshape(-1)[:NS]
    return pos, neg


# revision 8
# speedup vs baseline: 3.0660x; 1.0123x over previous
"""DGI (Deep Graph Infomax) Trainium2 kernel — v2.

Strategy (8 NeuronCores, one shared SPMD program):
  - Nodes sharded by destination: core c owns dst nodes [c*N/8, (c+1)*N/8).
  - Aggregate-then-multiply: z = PReLU((A_hat @ x) @ W + b).  The x-space
    aggregation needs no precomputed xw, so gathers start immediately.
  - Paired table tab[i] = [x[i] | x[perm[i]]] (f16, 1KB rows).  dma_gather
    cost is row-count-bound (~8ns/row, flat 256B..1KB), so one gather per
    edge fetches BOTH passes' source rows.  One shared weighted one-hot
    (fused is_equal+mult on DVE) and one PE matmul [128e,128d]^T@[128e,512]
    accumulate [agg1|agg2] per dst tile in PSUM.
  - Gathers round-robin over 4 SWDGE queues (~2x DMA overlap).
  - Epilogue per dst tile: PSUM->SBUF f16, 2+2 PE transposes, (aggT @ W)
    k-accumulated, bias+PReLU on DVE; z1/z2 tiles stored f16 in SBUF;
    z1 column-sums accumulate for the summary.
  - summary = sigmoid(mean over all z1) via ones-matmul column reduce +
    1KB AllReduce; wsum = disc_W @ summary on PE; broadcast via K=1 matmul.
  - pos/neg = z . wsum per tile on DVE; per-core [128, DT] outputs,
    host unshards.
"""

import os

import numpy as np

_P = 128
_LO = 32768
_C = 8


def _build_streams(sidx, ed, ew, C, NS, DT):
    """Per-core gather/weight/dstl streams with shared tile structure.

    Returns (idx_sbuf [C,128,n_et*8] i16, w_sbuf [C,128,n_et] f32,
             dl_sbuf [C,128,n_et] f32, Tmax [DT,2], off_tiles [DT,2], n_et)
    """
    core = ed // NS
    ldst = ed - core * NS
    dt = ldst // _P
    dstl = ldst % _P
    cls = (sidx >= _LO).astype(np.int64)

    gid = (core * DT + dt) * 2 + cls
    NG = C * DT * 2
    cnt = np.bincount(gid, minlength=NG).reshape(C, DT, 2)
    T = -(-cnt // _P)
    Tmax = T.max(axis=0)
    flat = Tmax.reshape(-1)
    off_tiles = np.concatenate([[0], np.cumsum(flat)[:-1]]).reshape(DT, 2)
    n_et = int(flat.sum())

    order = np.argsort(gid, kind="stable")
    sorted_gid = gid[order]
    g_starts = np.concatenate(
        [[0], np.cumsum(np.bincount(sorted_gid, minlength=NG))[:-1]]
    )
    rank = np.arange(order.size) - g_starts[sorted_gid]
    g_dt = (sorted_gid // 2) % DT
    g_cls = sorted_gid % 2
    pos = off_tiles[g_dt, g_cls] * _P + rank
    core_s = sorted_gid // (DT * 2)

    L = n_et * _P
    idx16 = np.zeros((C, L), np.int16)
    wv = np.zeros((C, L), np.float32)
    dl = np.full((C, L), -1.0, np.float32)
    sidx_s = sidx[order]
    idx16[core_s, pos] = (sidx_s - g_cls * _LO).astype(np.int16)
    wv[core_s, pos] = ew[order]
    dl[core_s, pos] = dstl[order].astype(np.float32)

    idx_w = idx16.reshape(C, L // 16, 16).transpose(0, 2, 1)
    idx_sbuf = np.ascontiguousarray(np.tile(idx_w, (1, 8, 1)))
    w_sbuf = np.ascontiguousarray(wv.reshape(C, n_et, _P).transpose(0, 2, 1))
    dl_sbuf = np.ascontiguousarray(dl.reshape(C, n_et, _P).transpose(0, 2, 1))
    return idx_sbuf, w_sbuf, dl_sbuf, Tmax, off_tiles, n_et


def kernel(x, W, b, a, disc_W, edge_index, perm):
    import bass_rust
    import concourse.bacc as bacc
    import concourse.mybir as mybir
    import concourse.tile as tile
    from concourse.bass_utils import run_bass_kernel_spmd

    x = np.asarray(x)
    W = np.asarray(W)
    b = np.asarray(b, np.float32)
    a = np.asarray(a, np.float32)
    disc_W = np.asarray(disc_W, np.float32)
    ei = np.asarray(edge_index, np.int64)
    perm_np = np.asarray(perm, np.int64)

    N, F = x.shape
    H = W.shape[1]
    C = _C
    NS = N // C
    DT = -(-NS // _P)
    LAST = NS - (DT - 1) * _P
    F2 = 2 * F  # paired row width
    f16 = mybir.dt.float16
    f32 = mybir.dt.float32
    NQ = 4  # SWDGE queues
    USE_ACT = os.environ.get("KV_ACT", "1") == "1"
    USE_TTR = os.environ.get("KV_TTR", "1") == "1"
    GBUFS = int(os.environ.get("KV_GBUFS", "3"))

    # ---- host preprocessing -------------------------------------------
    src = ei[0]
    dst = ei[1]
    deg = (np.bincount(dst, minlength=N) + 1.0).astype(np.float32)
    dinv = (1.0 / np.sqrt(deg)).astype(np.float32)
    loops = np.arange(N, dtype=np.int64)
    es = np.concatenate([src, loops])
    ed = np.concatenate([dst, loops])
    ew = dinv[es] * dinv[ed]

    i1, w1, d1, T1, O1, n_et = _build_streams(es, ed, ew, C, NS, DT)
    a_val = float(np.asarray(a).reshape(-1)[0])

    x_f16 = x.astype(np.float16)
    tab = np.ascontiguousarray(np.concatenate([x_f16, x_f16[perm_np]], axis=1))
    W_f16 = np.ascontiguousarray(W.astype(np.float16))
    dwT = np.ascontiguousarray(disc_W.T.astype(np.float32))
    iota_np = np.tile(np.arange(_P, dtype=np.float16)[None, :], (_P, 1))
    ident16_np = np.eye(_P, dtype=np.float16)

    max_Tl = max(int(T1[:, 0].max()), 1)
    max_Th = max(int(T1[:, 1].max()), 1)

    # ---- device program -----------------------------------------------
    nc = bacc.Bacc(
        "TRN2", target_bir_lowering=False, debug=False, num_devices=C,
        num_swdge_queues=NQ,
    )

    t_tab = nc.dram_tensor("tab", [N, F2], f16, kind="ExternalInput")
    t_W = nc.dram_tensor("w16", [F, H], f16, kind="ExternalInput")
    t_b = nc.dram_tensor("bvec", [H], f32, kind="ExternalInput")
    t_a = nc.dram_tensor("avec", [1], f32, kind="ExternalInput")
    t_dwT = nc.dram_tensor("dwT", [H, H], f32, kind="ExternalInput")
    t_iota = nc.dram_tensor("iota", [_P, _P], f16, kind="ExternalInput")
    t_ident = nc.dram_tensor("ident_in", [_P, _P], f32, kind="ExternalInput")
    t_ident16 = nc.dram_tensor("ident16", [_P, _P], f16, kind="ExternalInput")
    t_i1 = nc.dram_tensor("idx1", [_P, n_et * 8], mybir.dt.int16, kind="ExternalInput")
    t_w1 = nc.dram_tensor("wgt1", [_P, n_et], f32, kind="ExternalInput")
    t_d1 = nc.dram_tensor("dstl1", [_P, n_et], f32, kind="ExternalInput")

    t_pos = nc.dram_tensor("pos_out", [_P, DT], f32, kind="ExternalOutput")
    t_neg = nc.dram_tensor("neg_out", [_P, DT], f32, kind="ExternalOutput")

    t_ar_in = nc.dram_tensor("ar_in", [H], f32)
    t_ar_out = nc.dram_tensor("ar_out", [H], f32, addr_space="Shared")

    tab_lo = t_tab[0:_LO, :]
    tab_hi = t_tab[_LO:N, :]

    with tile.TileContext(nc) as tc:
        import contextlib

        ctx = contextlib.ExitStack()
        consts = ctx.enter_context(tc.tile_pool(name="consts", bufs=1))
        glo = ctx.enter_context(tc.tile_pool(name="glo", bufs=GBUFS))
        ghi = ctx.enter_context(tc.tile_pool(name="ghi", bufs=GBUFS))
        stp = ctx.enter_context(tc.tile_pool(name="stp", bufs=4))
        aggps = ctx.enter_context(tc.tile_pool(name="aggps", bufs=2, space="PSUM"))
        trps = ctx.enter_context(tc.tile_pool(name="trps", bufs=2, space="PSUM"))
        zps = ctx.enter_context(tc.tile_pool(name="zps", bufs=2, space="PSUM"))
        ep = ctx.enter_context(tc.tile_pool(name="ep", bufs=3))
        misc = ctx.enter_context(tc.tile_pool(name="misc", bufs=2))
        miscps = ctx.enter_context(tc.tile_pool(name="miscps", bufs=1, space="PSUM"))

        # ---- constants ----
        W0 = consts.tile([_P, H], f16, tag="W0")
        W1 = consts.tile([_P, H], f16, tag="W1")
        nc.sync.dma_start(W0[:], t_W[0:_P, :])
        nc.sync.dma_start(W1[:], t_W[_P : 2 * _P, :])
        iota_t = consts.tile([_P, _P], f16, tag="iota")
        nc.sync.dma_start(iota_t[:], t_iota[:])
        ident16 = consts.tile([_P, _P], f16, tag="ident16")
        nc.sync.dma_start(ident16[:], t_ident16[:])
        b_sb = consts.tile([1, H], f32, tag="b_sb")
        nc.sync.dma_start(b_sb[:], t_b[None, :])
        a_sb = consts.tile([1, 1], f32, tag="a_sb")
        nc.sync.dma_start(a_sb[:], t_a[None, :])
        dwT0 = consts.tile([_P, H], f32, tag="dwT0")
        dwT1 = consts.tile([_P, H], f32, tag="dwT1")
        nc.sync.dma_start(dwT0[:], t_dwT[0:_P, :])
        nc.sync.dma_start(dwT1[:], t_dwT[_P : 2 * _P, :])
        ones_row = consts.tile([1, _P], f32, tag="ones_row")
        nc.vector.memset(ones_row[:], 1.0)
        ones_col = consts.tile([_P, 1], f32, tag="ones_col")
        nc.vector.memset(ones_col[:], 1.0)

        # broadcasts via K=1 matmul
        bb_ps = miscps.tile([_P, H], f32, tag="mps")
        nc.tensor.matmul(bb_ps[:], ones_row[:], b_sb[:], start=True, stop=True)
        b_bc = consts.tile([_P, H], f32, tag="b_bc")
        nc.vector.tensor_copy(b_bc[:], bb_ps[:])
        ab_ps = miscps.tile([_P, 1], f32, tag="mps")
        nc.tensor.matmul(ab_ps[:], ones_row[:], a_sb[:], start=True, stop=True)
        a_bc = consts.tile([_P, 1], f32, tag="a_bc")
        nc.vector.tensor_copy(a_bc[:], ab_ps[:])

        # ---- stream loads ----
        i1_sb = consts.tile([_P, n_et * 8], mybir.dt.int16, tag="i1")
        w1_sb = consts.tile([_P, n_et], f32, tag="w1")
        d1_sb = consts.tile([_P, n_et], f32, tag="d1")
        nc.sync.dma_start(i1_sb[:], t_i1[:])
        nc.sync.dma_start(w1_sb[:], t_w1[:])
        nc.sync.dma_start(d1_sb[:], t_d1[:])

        # ---- persistent z tiles + summary accumulator ----
        zbuf1 = consts.tile([_P, DT * H], f16, tag="zbuf1")
        zbuf2 = consts.tile([_P, DT * H], f16, tag="zbuf2")
        cacc = consts.tile([_P, H], f32, tag="cacc")
        nc.vector.memset(cacc[:], 0.0)

        qctr = [0]

        # ---- main sweep: one pass over dst tiles serves both encodings --
        for dti in range(DT):
            Tl, Th = int(T1[dti, 0]), int(T1[dti, 1])
            gl = gh = None
            if Tl:
                o = int(O1[dti, 0])
                gl = glo.tile([_P, max_Tl, F2], f16, tag="gl")
                nc.gpsimd.dma_gather(
                    gl[:, :Tl, :],
                    tab_lo,
                    i1_sb[:, 8 * o : 8 * (o + Tl)],
                    Tl * _P,
                    Tl * _P,
                    F2,
                    single_packet=(Tl * _P <= 1024),
                    queue_num=qctr[0] % NQ,
                )
                qctr[0] += 1
            if Th:
                o = int(O1[dti, 1])
                gh = ghi.tile([_P, max_Th, F2], f16, tag="gh")
                nc.gpsimd.dma_gather(
                    gh[:, :Th, :],
                    tab_hi,
                    i1_sb[:, 8 * o : 8 * (o + Th)],
                    Th * _P,
                    Th * _P,
                    F2,
                    single_packet=(Th * _P <= 1024),
                    queue_num=qctr[0] % NQ,
                )
                qctr[0] += 1

            ps = aggps.tile([_P, F2], f32, tag="aggps")
            n_mm = Tl + Th
            k = 0
            for Tn, g, o0 in ((Tl, gl, int(O1[dti, 0])), (Th, gh, int(O1[dti, 1]))):
                for j in range(Tn):
                    t = o0 + j
                    stw = stp.tile([_P, _P], f16, tag="stw")
                    nc.vector.tensor_scalar(
                        stw[:],
                        iota_t[:],
                        d1_sb[:, t : t + 1],
                        w1_sb[:, t : t + 1],
                        mybir.AluOpType.is_equal,
                        mybir.AluOpType.mult,
                    )
                    nc.tensor.matmul(
                        ps[:],
                        stw[:],
                        g[:, j, :],
                        start=(k == 0),
                        stop=(k == n_mm - 1),
                    )
                    k += 1

            # epilogue: agg [128, 512] = [agg1 | agg2] -> z1, z2 tiles
            agg_sb = ep.tile([_P, F2], f16, tag="agg_sb")
            nc.any.tensor_copy(agg_sb[:], ps[:])
            zp = zps.tile([_P, F2], f32, tag="zp")
            for half in range(2):
                nc.tensor.matmul(
                    zp[:, half * H : (half + 1) * H], ones_row[:], b_sb[:],
                    start=True, stop=False,
                )
            for k4 in range(4):
                tp = trps.tile([_P, _P], f16, tag="trps")
                nc.tensor.transpose(
                    tp[:], agg_sb[:, k4 * _P : (k4 + 1) * _P], ident16[:]
                )
                aggT = ep.tile([_P, _P], f16, tag="aggT")
                nc.any.tensor_copy(aggT[:], tp[:])
                half = 0 if k4 < 2 else 1
                Wk = W0 if (k4 % 2) == 0 else W1
                nc.tensor.matmul(
                    zp[:, half * H : (half + 1) * H], aggT[:], Wk[:],
                    start=False, stop=((k4 % 2) == 1),
                )

            if USE_ACT:
                # z1: PReLU on scalar engine -> f32 scratch (for cacc) + f16 store
                zf = misc.tile([_P, H], f32, tag="zf")
                nc.scalar.activation(
                    zf[:], zp[:, 0:H], mybir.ActivationFunctionType.Prelu,
                    alpha=a_val,
                )
                nc.any.tensor_copy(zbuf1[:, dti * H : (dti + 1) * H], zf[:])
                rows = LAST if dti == DT - 1 else _P
                nc.vector.tensor_tensor(
                    cacc[:rows, :], cacc[:rows, :], zf[:rows, :],
                    mybir.AluOpType.add,
                )
                # z2: PReLU straight to f16 store
                nc.scalar.activation(
                    zbuf2[:, dti * H : (dti + 1) * H], zp[:, H:F2],
                    mybir.ActivationFunctionType.Prelu, alpha=a_val,
                )
            else:
                for half, zbuf, is_z1 in ((0, zbuf1, True), (1, zbuf2, False)):
                    t1 = misc.tile([_P, H], f32, tag="t1")
                    nc.vector.tensor_scalar(
                        t1[:], zp[:, half * H : (half + 1) * H], 0.0, a_bc[:, 0:1],
                        mybir.AluOpType.min, mybir.AluOpType.mult,
                    )
                    t2 = misc.tile([_P, H], f32, tag="t2")
                    nc.vector.tensor_scalar(
                        t2[:], zp[:, half * H : (half + 1) * H], 0.0, None,
                        mybir.AluOpType.max,
                    )
                    zf = misc.tile([_P, H], f32, tag="zf")
                    nc.vector.tensor_tensor(zf[:], t1[:], t2[:], mybir.AluOpType.add)
                    nc.any.tensor_copy(zbuf[:, dti * H : (dti + 1) * H], zf[:])
                    if is_z1:
                        rows = LAST if dti == DT - 1 else _P
                        nc.vector.tensor_tensor(
                            cacc[:rows, :], cacc[:rows, :], zf[:rows, :],
                            mybir.AluOpType.add,
                        )

        # ---- summary + AllReduce ----
        cs_ps = miscps.tile([1, H], f32, tag="mps")
        nc.tensor.matmul(cs_ps[:], ones_col[:], cacc[:], start=True, stop=True)
        cs_sb = misc.tile([1, H], f32, tag="cs_sb")
        nc.vector.tensor_copy(cs_sb[:], cs_ps[:])
        nc.sync.dma_start(t_ar_in[None, :], cs_sb[:])
        nc.gpsimd.collective_compute(
            "AllReduce",
            mybir.AluOpType.add,
            replica_groups=[list(range(C))],
            ins=[t_ar_in[:]],
            outs=[t_ar_out[:]],
        )
        sums_sb = misc.tile([1, H], f32, tag="sums_sb")
        nc.sync.dma_start(sums_sb[:], t_ar_out[None, :])
        summ_sb = misc.tile([1, H], f32, tag="summ_sb")
        nc.scalar.activation(
            summ_sb[:], sums_sb[:], mybir.ActivationFunctionType.Sigmoid,
            scale=1.0 / N,
        )

        # ---- wsum = disc_W @ summary ----
        ident = consts.tile([_P, _P], f32, tag="ident")
        nc.sync.dma_start(ident[:], t_ident[:])
        sT = misc.tile([_P, 2], f32, tag="sT")
        for c_i in range(2):
            tp = miscps.tile([_P, _P], f32, tag="mps")
            nc.tensor.transpose(
                tp[:, 0:1],
                summ_sb[0:1, c_i * _P : (c_i + 1) * _P],
                ident[0:1, 0:1],
            )
            nc.vector.tensor_copy(sT[:, c_i : c_i + 1], tp[:, 0:1])
        ws_ps = miscps.tile([1, H], f32, tag="mps")
        nc.tensor.matmul(ws_ps[:], sT[:, 0:1], dwT0[:], start=True, stop=False)
        nc.tensor.matmul(ws_ps[:], sT[:, 1:2], dwT1[:], start=False, stop=True)
        ws_sb = misc.tile([1, H], f32, tag="ws_sb")
        nc.vector.tensor_copy(ws_sb[:], ws_ps[:])
        wb_ps = miscps.tile([_P, H], f32, tag="mps")
        nc.tensor.matmul(wb_ps[:], ones_row[:], ws_sb[:], start=True, stop=True)
        wsum_bc = consts.tile([_P, H], f16, tag="wsum_bc")
        nc.vector.tensor_copy(wsum_bc[:], wb_ps[:])

        # ---- pos/neg dots ----
        pos_acc = consts.tile([_P, DT], f32, tag="pos_acc")
        neg_acc = consts.tile([_P, DT], f32, tag="neg_acc")
        scratch = misc.tile([_P, H], f16, tag="scratch")
        for zbuf, acc in ((zbuf1, pos_acc), (zbuf2, neg_acc)):
            for dti in range(DT):
                if USE_TTR:
                    nc.vector.tensor_tensor_reduce(
                        scratch[:], zbuf[:, dti * H : (dti + 1) * H], wsum_bc[:],
                        1.0, 0.0,
                        mybir.AluOpType.mult, mybir.AluOpType.add,
                        acc[:, dti : dti + 1],
                    )
                else:
                    nc.vector.tensor_tensor(
                        scratch[:], zbuf[:, dti * H : (dti + 1) * H], wsum_bc[:],
                        mybir.AluOpType.mult,
                    )
                    nc.vector.reduce_sum(
                        acc[:, dti : dti + 1], scratch[:], bass_rust.AxisListType.X
                    )

        nc.sync.dma_start(t_pos[:], pos_acc[:])
        nc.sync.dma_start(t_neg[:], neg_acc[:])
        ctx.close()

    nc.compile()

    in_maps = []
    for c in range(C):
        in_maps.append(
            {
                "tab": tab,
                "w16": W_f16,
                "bvec": b,
                "avec": a,
                "dwT": dwT,
                "iota": iota_np,
                "ident_in": np.eye(_P, dtype=np.float32),
                "ident16": ident16_np,
                "idx1": i1[c],
                "wgt1": w1[c],
                "dstl1": d1[c],
            }
        )

    if os.environ.get("KERNEL_SIM", "0") == "1":
        from concourse import bass_interp

        sim = bass_interp.MultiCoreSim(nc, C)
        for c in range(C):
            for k, v in in_maps[c].items():
                sim.cores[c].tensor(k)[:] = v
        sim.simulate()
        results = [
            {
                "pos_out": np.array(sim.cores[c].tensor("pos_out")),
                "neg_out": np.array(sim.cores[c].tensor("neg_out")),
            }
            for c in range(C)
        ]
    else:
        trace = os.environ.get("KERNEL_TRACE", "0") == "1"
        kw = {}
        if trace:
            kw["trace"] = True
        res = run_bass_kernel_spmd(nc, in_maps, core_ids=list(range(C)), **kw)
        kernel.last_result = res
        results = res.results

    pos = np.zeros(N, np.float32)
    neg = np.zeros(N, np.float32)
    for c in range(C):
        pos[c * NS : (c + 1) * NS] = results[c]["pos_out"].T.reshape(-1)[:NS]
        neg[c * NS : (c + 1) * NS] = results[c]["neg_out"].T.reshape(-1)[:NS]
    return pos, neg
